# revision 15
# baseline (speedup 1.0000x reference)
"""AttMatrixCov loss kernel for 8 Trainium2 NeuronCores.

Math
----
Reference:
    loss = sum_{a, i<j} mean((attc[a,i] outer attc[a,j] - I_C)^2)
         + sum_{a, i<j} mean((atts[a,i]^T atts[a,j] - I_W)^2)

With A_t = S_t S_t^T ([H,H] Gram, contracting W) the pairwise sums
collapse:
    sum_{i<j} |S_i^T S_j|_F^2 = 1/2 (|M|_F^2 - sum_t |A_t|_F^2),  M = sum_t A_t
    sum_{i<j} tr(S_i^T S_j)   = 1/2 (|R|_F^2 - sum_t |S_t|_F^2),  R = sum_t S_t

The O(N^3) Gram work runs on device in fp8(e4m3) with DoubleRow
matmuls (2 K-planes per pass); everything the device produces is
reduced on-chip to per-partition scalars ([128,16] stats per core), so
output DMA is 8KB instead of the 768KB the previous version shipped.
The O(N^2)/O(N) glue (R-term, sum_t |S_t|^2, channel branch) is exact
f64 on host; fp8 only perturbs the Gram term (+~1.3e-3 rel, vs the
2e-2 gate).

Sharding: 8 cores = (natt=4) x (H row-block m=0,1). Each core loads
all 8 temps of S^T for its `a` as fp8 [128p, 8t, 2g, 256h] (512KB,
w=2p+g on partitions) and computes its 128-row block of every A_t and
of M:
    per t: one DoubleRow MM -> psA (A_t rows), one DoubleRow MM
    accumulating into psM (M rows), same LDWEIGHTS; DVE
    tensor_tensor_reduce squares psA into stats[:, t]; at the end psM
    is squared into stats[:, 8].
Host sums the 128-partition stats in f64 and combines.

Input DMA is 4x128KB chunks (t-pairs) alternating the two HWDGE
queues so the PE starts after the first chunk and streams behind the
DMA. No activation tables, no memsets, no warm-up: the first measured
instruction is the first input DMA.
"""

import numpy as np

NATT, NTEMP, C = 4, 8, 1024
H, W = 256, 256
NPAIR = NTEMP * (NTEMP - 1) // 2
P = 128
N_CORES = 8
STATS_COLS = 16

_nc_cache = None


def _build():
    import concourse.bacc as bacc
    import concourse.tile as tile
    from concourse import mybir

    f32 = mybir.dt.float32
    fp8 = mybir.dt.float8e4
    nc = bacc.Bacc(enable_partition_id=False)
    # sb[p, t, g, h] = atts[a, t, h, 2p+g]  (S^T, w on partitions)
    sb_in = nc.dram_tensor("sb", [P, NTEMP, 2, H], fp8, kind="ExternalInput")
    # One NEFF for all 8 cores: lhsT always reads sb columns 0:128.
    # m=1 cores get sb with the two 128-column halves swapped by the
    # host, so their lhsT holds S^T[:, 128:256] and they compute A/M
    # rows 128:256 (with rhs columns permuted, which squares and
    # row-block sums don't see).
    st_out = nc.dram_tensor("stats", [P, STATS_COLS], f32, kind="ExternalOutput")

    with tile.TileContext(nc) as tc:
        with (
            tc.tile_pool(name="sall", bufs=1) as sall,
            tc.tile_pool(name="acc", bufs=1) as accp,
            tc.tile_pool(name="scr", bufs=2) as scr,
            tc.tile_pool(name="ps_a", bufs=1, space="PSUM") as ps_a,
        ):
            ht = sall.tile([P, NTEMP, 2, H], fp8)
            # Input split over three DMA paths. GpSimd (SWDGE) clears
            # the framework preamble ~1us before sync/scalar, so it
            # carries the first temps; scalar's queue starts slowest
            # (ACT table load shares the engine) and gets the last.
            nc.gpsimd.dma_start(out=ht[:, 0:3], in_=sb_in[:, 0:3])
            nc.sync.dma_start(out=ht[:, 3:6], in_=sb_in[:, 3:6])
            nc.scalar.dma_start(out=ht[:, 6:8], in_=sb_in[:, 6:8])

            stats = accp.tile([P, STATS_COLS], f32)
            # One PSUM tensor holds all 8 A_t blocks (4 banks, one
            # pair per bank): the PE never waits on bank reuse, ACT
            # squares two quads ([128,4,256] each, one accumulator
            # read per quad), DVE folds M at pair granularity.
            # NOTE: nc.vector.tensor_tensor_reduce hangs this
            # device/runtime combo (isolated empirically) -- only ACT
            # Square+accum and plain DVE ops here.
            ps = ps_a.tile([P, NTEMP, H], f32)
            m2 = accp.tile([P, 2, H], f32)
            for t in range(NTEMP):
                nc.tensor.matmul(
                    ps[:, t],
                    lhsT=ht[:, t, :, 0:P],
                    rhs=ht[:, t],
                    start=True,
                    stop=True,
                    perf_mode=mybir.MatmulPerfMode.DoubleRow,
                )
                if t % 2 == 1:
                    pair = t // 2
                    if pair == 0:
                        nc.vector.tensor_copy(m2, ps[:, 0:2])
                    else:
                        nc.vector.tensor_add(
                            m2, m2, ps[:, 2 * pair : 2 * pair + 2]
                        )
                if t == 3 or t == NTEMP - 1:
                    q = t // 4
                    sq = scr.tile([P, 4, H], f32, tag="sq")
                    nc.scalar.activation(
                        out=sq,
                        in_=ps[:, 4 * q : 4 * q + 4],
                        func=mybir.ActivationFunctionType.Square,
                        accum_out=stats[:, q : q + 1],
                    )
            m_acc = accp.tile([P, H], f32)
            nc.vector.tensor_add(m_acc, m2[:, 0], m2[:, 1])
            sqm = scr.tile([P, H], f32, tag="sqm")
            nc.vector.tensor_mul(sqm, m_acc, m_acc)
            nc.vector.reduce_sum(stats[:, 2:3], sqm, axis=mybir.AxisListType.X)
            nc.sync.dma_start(out=st_out[:, :], in_=stats)
    nc.finalize()
    return nc


last_results = None


def _ensure_ntff_hook():
    """Register the axon NTFF profile hook if the image's antenv lacks it.

    Only matters when BASS_TRACE=1; harmless otherwise."""
    import sys
    import types

    try:
        import antenv.axon_hooks  # noqa: F401

        return
    except ImportError:
        pass
    try:
        from trn_agent_boot.trn_boot import _ntff_profile_via_ctypes

        hook = _ntff_profile_via_ctypes("/opt/axon/libaxon_pjrt.so")
    except Exception:
        hook = None
    mod = types.ModuleType("antenv.axon_hooks")
    mod.get_axon_ntff_profile_hook = lambda: hook
    mod.set_axon_ntff_profile_hook = lambda h: None
    sys.modules["antenv.axon_hooks"] = mod


def kernel(attc: np.ndarray, atts: np.ndarray) -> np.ndarray:
    global _nc_cache, last_results
    _ensure_ntff_hook()
    import ml_dtypes
    from concourse.bass_utils import run_bass_kernel_spmd

    if _nc_cache is None:
        _nc_cache = _build()
    nc = _nc_cache

    attc = np.asarray(attc)
    atts = np.asarray(atts)

    in_maps = []
    sb_cache = {}
    for core in range(N_CORES):
        a, m = core // 2, core % 2
        if (a, m) not in sb_cache:
            q = atts[a].astype(ml_dtypes.float8_e4m3fn)  # [8,256,256]
            # sb[p,t,g,h] = q[t, h, 2p+g]
            sb = np.ascontiguousarray(
                q.transpose(2, 0, 1).reshape(P, 2, NTEMP, H).transpose(0, 2, 1, 3)
            )
            # m=1 cores compute A rows 128:256: swap the two
            # 128-column halves so lhsT (always cols 0:128) holds
            # S^T[:, 128:256]. rhs columns come out permuted, which
            # is irrelevant for squares / row sums.
            sb_cache[(a, 0)] = sb
            sb_cache[(a, 1)] = np.ascontiguousarray(
                np.concatenate([sb[:, :, :, P:], sb[:, :, :, :P]], axis=3)
            )
        in_maps.append({"sb": sb_cache[(a, m)]})

    res = run_bass_kernel_spmd(nc, in_maps, core_ids=list(range(N_CORES)))
    last_results = res
    outs = res.results

    total = 0.0
    for a in range(NATT):
        st0 = outs[2 * a]["stats"].astype(np.float64)
        st1 = outs[2 * a + 1]["stats"].astype(np.float64)
        sumA = st0[:, 0:2].sum() + st1[:, 0:2].sum()
        M2 = st0[:, 2].sum() + st1[:, 2].sum()

        S = atts[a].astype(np.float64)  # [8,256,256]
        R = S.sum(0)
        T = (S * S).sum()
        loss_s = (
            0.5 * (M2 - sumA) - ((R * R).sum() - T) + NPAIR * W
        ) / (W * W)

        c = attc[a].astype(np.float64)  # [8,1024]
        n_t = (c * c).sum(1)
        v = c.sum(0)
        loss_c = (
            0.5 * (n_t.sum() ** 2 - (n_t * n_t).sum())
            - ((v * v).sum() - n_t.sum())
            + NPAIR * C
        ) / (C * C)
        total += loss_s + loss_c

    return np.array(total, dtype=np.float32)
